# revision 1
# baseline (speedup 1.0000x reference)
"""FourDirGradientConv + 1x1 compress + BatchNorm, Trainium2 Bass kernel.

Math: feat = concat_g(shift_g(x) - x), y = W @ feat, out = BN(y) * gamma + beta
with shifts g in {(-1,+1), (-1,-1), (+1,+1), (+1,-1)} (zero-padded).

Rewrite: y[o,i,j] = sum_g (Wg @ x)[o, i+di_g, j+dj_g] - (sum_g Wg @ x)[o,i,j]
so y is 5 PSUM-accumulated matmuls whose shifts are just rhs AP offsets into a
zero-padded SBUF x tile. Contraction K=32 is packed 4x via block-diagonal
weights (4 row-blocks of the image on the 128 partitions -> M=16 outputs =
4 rows x 4 channels), and 4x further via tile_position col groups.

Sharding: data-parallel over batch, core b <-> sample b. BN batch stats are a
[4,2] AllReduce across the 8 cores.
"""

import os
import numpy as np

import concourse.bass as bass
import concourse.tile as tile
import concourse.mybir as mybir
from concourse.bass_utils import run_bass_kernel_spmd

# problem constants (hardcoded per harness contract)
B, C, H, W = 8, 32, 512, 512
BN_EPS = 1e-5
N_CORES = 8

# tiling
P = 4  # partition blocks (row groups of the image)
R = 16  # rows per block per span
SPANS = H // (P * R)  # 8 spans of 64 rows
WP = W + 2  # padded row width in SBUF
NT_PART = 2 * R * W  # free size of y_store per partition (16384)

F32 = mybir.dt.float32
_DT_MAP = {
    "f32": mybir.dt.float32,
    "f16": mybir.dt.float16,
    "bf16": mybir.dt.bfloat16,
}
MM_DT = _DT_MAP[os.environ.get("BASSK_DT", "f16")]

# shift table: (di, dj) per group; group 4 = center with -sum(W) weights
SHIFTS = [(-1, +1), (-1, -1), (+1, +1), (+1, -1), (0, 0)]


def _split_multiwait(nc, max_waits=1):
    """Walrus here rejects >1 sync wait per instruction (tail Drain carries
    several); hoist extras onto same-engine NOPs placed just before."""
    for f in nc.m.functions:
        for b in f.blocks:
            insts = list(b.instructions)
            out = []
            changed = False
            for inst in insts:
                si = inst.sync_info
                if si is not None and len(si.on_wait) > max_waits:
                    waits = list(si.on_wait)
                    keep = waits[-max_waits:]
                    for k, wt in enumerate(waits[:-max_waits]):
                        out.append(
                            mybir.InstNoOp(
                                name=f"{inst.name}-waitsplit-{k}",
                                engine=inst.engine,
                                sync_info=mybir.SyncInfo(on_wait=[wt], on_update=[]),
                            )
                        )
                    inst.sync_info = mybir.SyncInfo(
                        on_wait=keep, on_update=list(si.on_update)
                    )
                    changed = True
                out.append(inst)
            if changed:
                b.instructions = out


def _mm(ap):
    return ap.bitcast(MM_DT) if MM_DT is not F32 else ap


def build_module():
    nc = bass.Bass(num_devices=N_CORES)

    xb = nc.declare_dram_parameter("xb", [C, H, W], F32, isOutput=False)
    wst = nc.declare_dram_parameter("wst", [5, 128, 16], F32, isOutput=False)
    sel = nc.declare_dram_parameter("sel", [128, 4], F32, isOutput=False)
    selbc = nc.declare_dram_parameter("selbc", [4, 128], F32, isOutput=False)
    gamma = nc.declare_dram_parameter("gamma", [4, 1], F32, isOutput=False)
    beta = nc.declare_dram_parameter("beta", [4, 1], F32, isOutput=False)
    y = nc.declare_dram_parameter("y", [4, H, W], F32, isOutput=True)

    with tile.TileContext(nc, num_cores=N_CORES) as tc:
        with (
            tc.tile_pool(name="xp", bufs=2) as xp,
            tc.tile_pool(name="const", bufs=1) as constp,
            tc.tile_pool(name="ystore", bufs=1) as ystp,
            tc.tile_pool(name="stats", bufs=1) as statsp,
            tc.tile_pool(name="small", bufs=1) as smallp,
            tc.tile_pool(name="ps", bufs=3, space="PSUM") as psp,
            tc.tile_pool(name="pss", bufs=1, space="PSUM") as pssp,
            tc.tile_pool(name="dram", bufs=1, space="DRAM") as dramp,
        ):
            # constants; x/w are cast to MM_DT by the gpsimd (SWDGE) DMA
            w_sb = constp.tile([128, 5, 16], MM_DT)
            if MM_DT is F32:
                nc.sync.dma_start(out=w_sb[:], in_=wst.transpose([1, 0, 2]))
            else:
                nc.gpsimd.dma_start(out=w_sb[:], in_=wst.transpose([1, 0, 2]))
            sel_sb = constp.tile([128, 4], F32)
            nc.sync.dma_start(out=sel_sb[:], in_=sel[:])
            selbc_sb = constp.tile([4, 128], F32)
            nc.sync.dma_start(out=selbc_sb[:], in_=selbc[:])
            gamma_sb = constp.tile([4, 1], F32)
            nc.sync.dma_start(out=gamma_sb[:], in_=gamma[:])
            beta_sb = constp.tile([4, 1], F32)
            nc.sync.dma_start(out=beta_sb[:], in_=beta[:])
            eps_sb = constp.tile([4, 1], F32)
            nc.gpsimd.memset(eps_sb[:], BN_EPS)

            # y_store: partition 32*Q + 4*p + o (Q = s%4), free slot
            # m = (s//4)*16 + 4*rr + j  ->  row r = 256*(s//4) + 64*Q + 16*p + 4*rr + j
            y_store = ystp.tile([128, NT_PART], F32)
            nc.gpsimd.memset(y_store[:], 0.0)

            xv3 = xb  # [C, H, W]

            for s in range(SPANS):
                x_t = xp.tile([128, R + 2, WP], MM_DT)
                # zero W pads (cols 0 and 513 of every row)
                nc.gpsimd.memset(x_t[:, :, 0:1], 0.0)
                nc.gpsimd.memset(x_t[:, :, WP - 1 : WP], 0.0)
                # load 4 blocks: block p holds rows [64s+16p-1, 64s+16p+17)
                for p in range(P):
                    r0 = 64 * s + 16 * p
                    lo, hi = r0 - 1, r0 + R + 1
                    dlo = 0
                    if lo < 0:
                        nc.gpsimd.memset(x_t[32 * p : 32 * p + 32, 0, :], 0.0)
                        lo, dlo = 0, 1
                    if hi > H:
                        nc.gpsimd.memset(
                            x_t[32 * p : 32 * p + 32, R + 1, :], 0.0
                        )
                        hi = H
                    xdst = x_t[32 * p : 32 * p + 32, dlo : dlo + (hi - lo), 1 : W + 1]
                    if MM_DT is F32:
                        nc.sync.dma_start(out=xdst, in_=xv3[:, lo:hi, :])
                    else:
                        nc.gpsimd.dma_start(out=xdst, in_=xv3[:, lo:hi, :])

                s2, Q = s // 4, s % 4
                for pair in range(2):
                    ps = psp.tile([128, 2, W], F32)  # 2 rr slots
                    for j in range(4):
                        for rr_h in range(2):
                            rr = 2 * pair + rr_h
                            lr = 4 * rr + j  # local row in every block
                            for g, (di, dj) in enumerate(SHIFTS):
                                nc.tensor.matmul(
                                    out=ps[32 * j : 32 * j + 16, rr_h, :],
                                    lhsT=w_sb[:, g, :],
                                    rhs=x_t[:, 1 + lr + di, 1 + dj : 1 + dj + W],
                                    start=(g == 0),
                                    stop=(g == 4),
                                    tile_position=(0, 32 * j),
                                )
                    # drain: dst slots m = 16*s2 + 8*pair + 4*rr_h + j
                    for j in range(4):
                        m0 = 16 * s2 + 8 * pair + j
                        dst = y_store[32 * Q : 32 * Q + 16, :].rearrange(
                            "q (m w) -> q m w", m=32
                        )[:, m0 : m0 + 5 : 4, :]
                        nc.scalar.copy(out=dst, in_=ps[32 * j : 32 * j + 16, :, :])

            # ---- BN stats: per-partition mean/var via bn_stats chunks ----
            stats = statsp.tile([128, NT_PART // 512, 6], F32)
            for k in range(NT_PART // 512):
                nc.vector.bn_stats(
                    out=stats[:, k, :], in_=y_store[:, 512 * k : 512 * k + 512]
                )
            mv = smallp.tile([128, 2], F32)
            nc.vector.bn_aggr(out=mv[:], in_=stats[:])

            # S12: col0 = mean_p, col1 = mean_p^2 + var_p  (scaled by NT_PART later)
            s12 = smallp.tile([128, 2], F32)
            nc.vector.tensor_copy(out=s12[:, 0:1], in_=mv[:, 0:1])
            nc.vector.tensor_tensor(
                out=s12[:, 1:2], in0=mv[:, 0:1], in1=mv[:, 0:1],
                op=mybir.AluOpType.mult,
            )
            nc.vector.tensor_tensor(
                out=s12[:, 1:2], in0=s12[:, 1:2], in1=mv[:, 1:2],
                op=mybir.AluOpType.add,
            )

            # combine over partitions: out[o, t] = sum_p sel[p,o] * s12[p,t]
            comb_ps = pssp.tile([4, 2], F32)
            nc.tensor.matmul(
                out=comb_ps[:], lhsT=sel_sb[:], rhs=s12[:], start=True, stop=True
            )
            comb = smallp.tile([4, 2], F32)
            nc.scalar.copy(out=comb[:], in_=comb_ps[:])

            # ---- AllReduce across cores ----
            cc_in = dramp.tile([4, 2], F32)
            cc_out = dramp.tile([4, 2], F32)
            nc.sync.dma_start(out=cc_in[:], in_=comb[:])
            nc.gpsimd.collective_compute(
                "AllReduce",
                mybir.AluOpType.add,
                replica_groups=[list(range(N_CORES))],
                ins=[cc_in.opt()],
                outs=[cc_out.opt()],
            )
            arin = smallp.tile([4, 2], F32)
            nc.sync.dma_start(out=arin[:], in_=cc_out[:])

            # ---- scale/bias math on [4,1] ----
            # mean = arin[:,0]/128 ; E[y^2] = arin[:,1]/128 (128 = NT/NT_PART)
            mean = smallp.tile([4, 1], F32)
            nc.scalar.mul(out=mean[:], in_=arin[:, 0:1], mul=1.0 / 128.0)
            var = smallp.tile([4, 1], F32)
            nc.scalar.mul(out=var[:], in_=arin[:, 1:2], mul=1.0 / 128.0)
            msq = smallp.tile([4, 1], F32)
            nc.vector.tensor_tensor(
                out=msq[:], in0=mean[:], in1=mean[:], op=mybir.AluOpType.mult
            )
            nc.vector.tensor_tensor(
                out=var[:], in0=var[:], in1=msq[:], op=mybir.AluOpType.subtract
            )
            # sd = sqrt(var + eps); rstd = 1/sd
            sd = smallp.tile([4, 1], F32)
            nc.scalar.activation(
                out=sd[:], in_=var[:], func=mybir.ActivationFunctionType.Sqrt,
                bias=eps_sb[:], scale=1.0,
            )
            rstd = smallp.tile([4, 1], F32)
            nc.vector.reciprocal(out=rstd[:], in_=sd[:])
            scbi = smallp.tile([4, 2], F32)
            nc.vector.tensor_tensor(
                out=scbi[:, 0:1], in0=gamma_sb[:], in1=rstd[:],
                op=mybir.AluOpType.mult,
            )
            # bias = beta - mean*scale
            tmp = smallp.tile([4, 1], F32)
            nc.vector.tensor_tensor(
                out=tmp[:], in0=mean[:], in1=scbi[:, 0:1], op=mybir.AluOpType.mult
            )
            nc.vector.tensor_tensor(
                out=scbi[:, 1:2], in0=beta_sb[:], in1=tmp[:],
                op=mybir.AluOpType.subtract,
            )
            # broadcast to [128, 2]: out[p, t] = scbi[p % 4, t] (via selbc)
            bc_ps = pssp.tile([128, 2], F32)
            nc.tensor.matmul(
                out=bc_ps[:], lhsT=selbc_sb[:], rhs=scbi[:], start=True, stop=True
            )
            scv = smallp.tile([128, 2], F32)
            nc.scalar.copy(out=scv[:], in_=bc_ps[:])

            # ---- affine + store out ----
            HALF = NT_PART // 2
            for s2 in range(2):
                nc.vector.tensor_scalar(
                    out=y_store[:, s2 * HALF : (s2 + 1) * HALF],
                    in0=y_store[:, s2 * HALF : (s2 + 1) * HALF],
                    scalar1=scv[:, 0:1],
                    scalar2=scv[:, 1:2],
                    op0=mybir.AluOpType.mult,
                    op1=mybir.AluOpType.add,
                )
                # row r = 256*s2 + 64*Q + 16*p + m ; partition = 32Q+4p+o
                y_r = y.rearrange(
                    "o (a q p r) w -> q p o a (r w)", a=2, q=4, p=4, r=16
                )
                for Q in range(4):
                    for p in range(P):
                        src = y_store[
                            32 * Q + 4 * p : 32 * Q + 4 * p + 4,
                            s2 * HALF : (s2 + 1) * HALF,
                        ]
                        nc.sync.dma_start(out=y_r[Q, p, :, s2, :], in_=src)

    _split_multiwait(nc)
    return nc


def _host_constants(w_compress):
    wst = np.zeros((5, 128, 16), dtype=np.float32)
    wsum = np.zeros((4, 32), dtype=np.float32)
    for g in range(4):
        wg = w_compress[:, 32 * g : 32 * g + 32]  # [4, 32] (o, c)
        wsum += wg
        for p in range(P):
            wst[g, 32 * p : 32 * p + 32, 4 * p : 4 * p + 4] = wg.T
    for p in range(P):
        wst[4, 32 * p : 32 * p + 32, 4 * p : 4 * p + 4] = -wsum.T

    sel = np.zeros((128, 4), dtype=np.float32)
    for prt in range(128):
        if prt % 32 < 16:
            sel[prt, prt % 4] = 1.0
    selbc = np.zeros((4, 128), dtype=np.float32)
    for prt in range(128):
        selbc[prt % 4, prt] = 1.0
    return wst, sel, selbc


_NC_CACHE = {}


def kernel(x, w_compress, gamma, beta):
    x = np.ascontiguousarray(np.asarray(x, dtype=np.float32))
    w_compress = np.asarray(w_compress, dtype=np.float32)
    gamma = np.asarray(gamma, dtype=np.float32)
    beta = np.asarray(beta, dtype=np.float32)

    if "nc" not in _NC_CACHE:
        _NC_CACHE["nc"] = build_module()
    nc = _NC_CACHE["nc"]

    wst, sel, selbc = _host_constants(w_compress)
    in_maps = []
    for b in range(B):
        in_maps.append(
            {
                "xb": np.ascontiguousarray(x[b]),
                "wst": wst,
                "sel": sel,
                "selbc": selbc,
                "gamma": gamma.reshape(4, 1),
                "beta": beta.reshape(4, 1),
            }
        )
    res = run_bass_kernel_spmd(
        nc,
        in_maps,
        core_ids=list(range(N_CORES)),
        trace=os.environ.get("BASSK_TRACE", "0") == "1",
    )
    _NC_CACHE["last_result"] = res
    out = np.stack([res.results[b]["y"] for b in range(B)], axis=0)
    return out

